# revision 1
# baseline (speedup 1.0000x reference)
"""Trainium2 Bass kernel for nn_Encoder_Block (B=2,S=2048,D=1024,H=16,FF=4096).

Sharding: 8 cores, core c -> (batch b=c//4, query block q=c%4 of 512 tokens).
Each core recomputes K/V for its whole batch (no cross-core collectives),
everything else is perfectly sharded. Host does transposes and gather.

Device layout: activations kept transposed [feature, token] throughout, so
every matmul in the chain is a natural lhsT/rhs pair with K=128 contraction
chunks and N=512 moving dim. Attention computes transposed scores [t, sq];
softmax normalizer rides along the PV matmul as a ones-column in V (M=65).
Masking + 1/sqrt(dh) scaling are folded into the Exp activation (bias/scale).
No max-subtraction: scores are O(1) by construction, exp is safe in fp32.
"""
import sys, types, os
sys.path.insert(0, "/opt/trn_rl_repo")
import numpy as np
from contextlib import ExitStack

import concourse.bass as bass
import concourse.tile as tile
from concourse import bacc, mybir
from concourse.bass_utils import run_bass_kernel_spmd

B, S, D, H, FF = 2, 2048, 1024, 16, 4096
DH = D // H            # 64
SQ = 512               # query tokens per core
NCORES = 8
NSC = 4                # super-chunks over S (512 keys each)
NTC = 4                # 128-token t-chunks per super-chunk
EPS = 1e-5
MASK_NEG = -60.0       # exp(-60) ~ 8.8e-27 => masked keys contribute ~0

F32 = mybir.dt.float32
# PE compute dtype for matmul-fed tensors:
#   bf16 : 1 cyc/row, half DMA/LDWEIGHTS cost, ~2e-3 output error
#   f32r : 1 cyc/row TF32-like, ~3.5e-4 output error
_MODE = os.environ.get("KERNEL_MM_DT", "f32r")
DT = {"f32r": mybir.dt.float32r, "f32": mybir.dt.float32,
      "bf16": mybir.dt.bfloat16}[_MODE]


BF16 = mybir.dt.bfloat16


def _f(ap):
    # f32r tiles must be bitcast to f32 for DVE/ACT reads; bf16 is native
    if DT == mybir.dt.float32r:
        return ap.bitcast(F32)
    return ap


def _install_ntff_hook():
    """The image's antenv lacks axon_hooks; shim it so trace=True works."""
    try:
        import antenv.axon_hooks  # noqa
        return
    except ImportError:
        pass
    try:
        from trn_agent_boot.trn_boot import _ntff_profile_via_ctypes
        import antenv
        mod = types.ModuleType("antenv.axon_hooks")
        hook = _ntff_profile_via_ctypes("/opt/axon/libaxon_pjrt.so")
        mod.get_axon_ntff_profile_hook = lambda: hook
        mod.set_axon_ntff_profile_hook = lambda h: None
        sys.modules["antenv.axon_hooks"] = mod
        antenv.axon_hooks = mod
    except Exception:
        pass


def _mm(nc, out, lhsT, rhs, start, stop, tile_position=None):
    nc.tensor.matmul(out, lhsT, rhs,
                     start=start, stop=stop, tile_position=tile_position)


def build_nc():
    nc = bacc.Bacc(trn_type="TRN2", target_bir_lowering=False, debug=False,
                   num_devices=NCORES, dynamic_dma_scratch_size=512)
    AF = mybir.ActivationFunctionType
    OP = mybir.AluOpType

    # ---- DRAM I/O (per-core; program identical across cores) ----
    d_xT = nc.dram_tensor("xT", [D, S], DT, kind="ExternalInput")
    d_xq = nc.dram_tensor("xq", [D, SQ], DT, kind="ExternalInput")
    d_mask = nc.dram_tensor("maskb", [128, S // 128], F32, kind="ExternalInput")
    d_wq = nc.dram_tensor("wq", [D, D], DT, kind="ExternalInput")
    d_wk = nc.dram_tensor("wk", [D, D], DT, kind="ExternalInput")
    d_wv = nc.dram_tensor("wv", [D, D], DT, kind="ExternalInput")
    d_wo = nc.dram_tensor("wo", [D, D], DT, kind="ExternalInput")
    d_aw1 = nc.dram_tensor("aw1", [D, D], DT, kind="ExternalInput")
    d_aw2 = nc.dram_tensor("aw2", [D, D], DT, kind="ExternalInput")
    d_fw1 = nc.dram_tensor("fw1", [D, FF], DT, kind="ExternalInput")
    d_fw2 = nc.dram_tensor("fw2", [FF, D], DT, kind="ExternalInput")
    d_b1 = nc.dram_tensor("b1c", [128, 8], F32, kind="ExternalInput")
    d_g1 = nc.dram_tensor("g1c", [128, 8], F32, kind="ExternalInput")
    d_bb1 = nc.dram_tensor("bb1c", [128, 8], F32, kind="ExternalInput")
    d_fb1 = nc.dram_tensor("fb1c", [128, 32], F32, kind="ExternalInput")
    d_fb2 = nc.dram_tensor("fb2c", [128, 8], F32, kind="ExternalInput")
    d_b2 = nc.dram_tensor("b2c", [128, 8], F32, kind="ExternalInput")
    d_g2 = nc.dram_tensor("g2c", [128, 8], F32, kind="ExternalInput")
    d_bb2 = nc.dram_tensor("bb2c", [128, 8], F32, kind="ExternalInput")
    d_out = nc.dram_tensor("out", [D, SQ], F32, kind="ExternalOutput")

    r_xT = d_xT.ap().rearrange("(c p) s -> p c s", p=128)     # [128, 8, S]
    r_xq = d_xq.ap().rearrange("(c p) s -> p c s", p=128)     # [128, 8, SQ]
    r_wq = d_wq.ap().rearrange("(c p) n -> p c n", p=128)
    r_wk = d_wk.ap().rearrange("(c p) n -> p c n", p=128)
    r_wv = d_wv.ap().rearrange("(c p) n -> p c n", p=128)
    r_wo = d_wo.ap().rearrange("(c p) n -> p c n", p=128)
    r_aw1 = d_aw1.ap().rearrange("(c p) n -> p c n", p=128)
    r_aw2 = d_aw2.ap().rearrange("(c p) n -> p c n", p=128)
    r_fw1 = d_fw1.ap().rearrange("(c p) n -> p c n", p=128)   # [128, 8, FF]
    r_fw2 = d_fw2.ap().rearrange("(c p) n -> p c n", p=128)   # [128, 32, D]
    r_out = d_out.ap().rearrange("(c p) s -> p c s", p=128)

    with tile.TileContext(nc) as tc:
      with ExitStack() as top:
        # one packed const tile (tiles pad to 4KB/partition each otherwise):
        # cols 0:16 maskbias, 16:80 ones, 80:208 sel_e, 208:336 sel_o
        const = top.enter_context(tc.tile_pool(name="const", bufs=1))
        cst = const.tile([128, 336], F32, name="cst")
        mask_sb = cst[:, 0:16]
        ones_f = cst[:, 16:80]
        sel_e = cst[:, 80:208]
        sel_o = cst[:, 208:336]
        nc.sync.dma_start(mask_sb, d_mask.ap())
        nc.vector.memset(cst[:, 16:336], 0.0)
        nc.vector.memset(ones_f, 1.0)
        nc.vector.memset(sel_e[0:1, 0:64], 1.0)
        nc.vector.memset(sel_e[32:33, 64:128], 1.0)
        nc.vector.memset(sel_o[64:65, 0:64], 1.0)
        nc.vector.memset(sel_o[96:97, 64:128], 1.0)
        ones_sb = const.tile([128, 1], DT, name="ones")
        nc.vector.tensor_copy(ones_sb[:], ones_f[:, 0:1])

        p_x1 = top.enter_context(tc.tile_pool(name="px1", bufs=1))

        def layernorm_block(st, src_sb, gc, bc, res_sb, dst_sb, pref):
            """dst = LN(src) * g + b + res, all [128, 8, SQ] chunked over D."""
            pln = st.enter_context(tc.tile_pool(name=pref + "ln", bufs=1))
            pps = st.enter_context(tc.tile_pool(name=pref + "lps", bufs=1, space="PSUM"))
            sq_sb = pln.tile([128, 8, SQ], DT, name=pref + "sq")
            for d in range(8):
                nc.vector.tensor_mul(sq_sb[:, d, :], _f(src_sb[:, d, :]),
                                     _f(src_sb[:, d, :]))
            ps_s = pps.tile([1, SQ], F32, name=pref + "ps_s")
            ps_q = pps.tile([1, SQ], F32, name=pref + "ps_q")
            for d in range(8):
                _mm(nc, ps_s[:], ones_sb[:], src_sb[:, d, :],
                    start=(d == 0), stop=(d == 7))
            for d in range(8):
                _mm(nc, ps_q[:], ones_sb[:], sq_sb[:, d, :],
                    start=(d == 0), stop=(d == 7))
            mu = pln.tile([1, SQ], F32, name=pref + "mu")
            nc.scalar.mul(mu[:], ps_s[:], 1.0 / D)
            msq = pln.tile([1, SQ], F32, name=pref + "msq")
            nc.scalar.mul(msq[:], ps_q[:], 1.0 / D)
            var = pln.tile([1, SQ], F32, name=pref + "var")
            nc.vector.tensor_mul(var[:], mu[:], mu[:])
            nc.vector.tensor_sub(var[:], msq[:], var[:])
            nc.vector.tensor_scalar_add(var[:], var[:], EPS)
            # rstd = exp(-0.5*ln(var)): Ln/Exp share the ACT table set with
            # the attention exps (no sqrt-table load).
            lnv = pln.tile([1, SQ], F32, name=pref + "lnv")
            nc.scalar.activation(lnv[:], var[:], AF.Ln)
            rstd = pln.tile([1, SQ], F32, name=pref + "rstd")
            nc.scalar.activation(rstd[:], lnv[:], AF.Exp, scale=-0.5)
            mub = pln.tile([128, SQ], F32, name=pref + "mub")
            rsb = pln.tile([128, SQ], F32, name=pref + "rsb")
            nc.gpsimd.partition_broadcast(mub[:], mu[:])
            nc.gpsimd.partition_broadcast(rsb[:], rstd[:])
            tmp = pln.tile([128, 8, SQ], F32, name=pref + "tmp")
            for d in range(8):
                nc.vector.tensor_sub(tmp[:, d, :], _f(src_sb[:, d, :]), mub[:])
                nc.vector.tensor_mul(tmp[:, d, :], tmp[:, d, :], rsb[:])
                nc.vector.tensor_scalar(tmp[:, d, :], tmp[:, d, :],
                                        gc[:, d:d + 1], bc[:, d:d + 1],
                                        OP.mult, OP.add)
                nc.vector.tensor_add(dst_sb[:, d, :], tmp[:, d, :],
                                     _f(res_sb[:, d, :]))

        # ============ Stages 1-3 share one scope: attention weights are
        # ============ tag-reused for the post-attention weights so their
        # ============ DMAs overlap the ACT-bound attention phase.
        with ExitStack() as s13:
            p_acc = s13.enter_context(tc.tile_pool(name="acc", bufs=1))
            acc = p_acc.tile([128, 8, SQ], DT, name="acc")
            # softmax denominators at partition 32*(h%4), free idx h//4;
            # init 1.0 so unused rows stay finite through reciprocal+selector
            nrm = p_acc.tile([128, 4, SQ], F32, name="nrm")
            nc.vector.memset(nrm[:], 1.0)

            pwkv = s13.enter_context(tc.tile_pool(name="pwkv", bufs=1))
            wk_sb = pwkv.tile([128, 8, D], DT, name="wk", tag="wk")
            wv_sb = pwkv.tile([128, 8, D], DT, name="wv", tag="wv")
            for d in range(8):
                nc.sync.dma_start(wk_sb[:, d, :], r_wk[:, d, :])
                nc.sync.dma_start(wv_sb[:, d, :], r_wv[:, d, :])
            pxsc = s13.enter_context(tc.tile_pool(name="pxsc", bufs=1))

            with ExitStack() as A:
                p_qT = A.enter_context(tc.tile_pool(name="qT", bufs=1))
                qT = p_qT.tile([128, 8, SQ], DT, name="qT")

                # ---- Stage 1a: Q^T projection ----
                with ExitStack() as st:
                    pw = st.enter_context(tc.tile_pool(name="pwq", bufs=1))
                    px = st.enter_context(tc.tile_pool(name="pxq", bufs=1))
                    pp = st.enter_context(tc.tile_pool(name="ppq", bufs=2, space="PSUM"))
                    wq_sb = pw.tile([128, 8, D], DT, name="wq")
                    xq_sb = px.tile([128, 8, SQ], DT, name="xqp")
                    for d in range(8):
                        nc.sync.dma_start(wq_sb[:, d, :], r_wq[:, d, :])
                        nc.sync.dma_start(xq_sb[:, d, :], r_xq[:, d, :])
                    for p in range(8):
                        ps = pp.tile([128, SQ], F32, name="psq")
                        for d in range(8):
                            _mm(nc, ps[:], wq_sb[:, d, p * 128:(p + 1) * 128],
                                xq_sb[:, d, :], start=(d == 0), stop=(d == 7))
                        nc.scalar.copy(qT[:, p, :], ps[:])

                # ---- Stage 1b+2: K/V proj + attention, flash over 4 sc ----
                pkv = A.enter_context(tc.tile_pool(name="pkv", bufs=2))
                pexp = A.enter_context(tc.tile_pool(name="pexp", bufs=3))
                aps = A.enter_context(ExitStack())
                psc = aps.enter_context(tc.tile_pool(name="psc", bufs=2, space="PSUM"))
                pp = psc
                ppv = aps.enter_context(tc.tile_pool(name="ppv", bufs=2, space="PSUM"))

                for sc in range(NSC):
                    t0 = sc * 512
                    xs = pxsc.tile([128, 8, 512], DT, name="xsc", tag="xsc")
                    for d in range(8):
                        nc.sync.dma_start(xs[:, d, :], r_xT[:, d, t0:t0 + 512])

                    kT = pkv.tile([128, 8, 512], DT, name="kT")
                    for p in range(8):
                        ps = pp.tile([128, 2, SQ], F32, name="s01")[:, 0, :]
                        for d in range(8):
                            _mm(nc, ps, wk_sb[:, d, p * 128:(p + 1) * 128],
                                xs[:, d, :], start=(d == 0), stop=(d == 7))
                        nc.scalar.copy(kT[:, p, :], ps)

                    vt = pkv.tile([128, NTC, 16, 65], DT, name="vt")
                    nc.vector.tensor_copy(
                        vt[:, :, :, 64:65],
                        ones_f.rearrange("p (a b c) -> p a b c", a=NTC, b=16))
                    for i in range(NTC):
                        for nb in range(2):
                            ps = pp.tile([128, 2, SQ], F32, name="s01")[:, 0, :]
                            for d in range(8):
                                _mm(nc, ps, xs[:, d, i * 128:(i + 1) * 128],
                                    wv_sb[:, d, nb * 512:(nb + 1) * 512],
                                    start=(d == 0), stop=(d == 7))
                            nc.scalar.copy(
                                vt[:, i, nb * 8:(nb + 1) * 8, 0:64],
                                ps.rearrange("p (h e) -> p h e", e=64))

                    for p in range(8):
                        h0, h1 = 2 * p, 2 * p + 1
                        pva = ppv.tile([128, 2, SQ], F32, name="pva")
                        for i in range(NTC):
                            tci = sc * NTC + i
                            s01 = psc.tile([128, 2, SQ], F32, name="s01")
                            _mm(nc, s01[:, 0, :],
                                kT[0:64, p, i * 128:(i + 1) * 128],
                                qT[0:64, p, :], start=True, stop=True,
                                tile_position=(0, 0))
                            _mm(nc, s01[:, 1, :],
                                kT[64:128, p, i * 128:(i + 1) * 128],
                                qT[64:128, p, :], start=True, stop=True,
                                tile_position=(64, 0))
                            e01 = pexp.tile([128, 2, SQ], DT, name="e01")
                            nc.scalar.activation(e01[:], s01[:], AF.Exp,
                                                 bias=mask_sb[:, tci:tci + 1],
                                                 scale=0.125)
                            _mm(nc, pva[0:65, 0, :], vt[:, i, h0, :], e01[:, 0, :],
                                start=(i == 0), stop=(i == NTC - 1))
                            _mm(nc, pva[0:65, 1, :], vt[:, i, h1, :], e01[:, 1, :],
                                start=(i == 0), stop=(i == NTC - 1))
                        a0, c0 = 32 * (h0 % 4), h0 // 4
                        a1, c1 = 32 * (h1 % 4), h1 // 4
                        if sc == 0:
                            nc.vector.tensor_copy(acc[0:64, p, :], pva[0:64, 0, :])
                            nc.vector.tensor_copy(acc[64:128, p, :], pva[0:64, 1, :])
                            nc.vector.tensor_copy(nrm[a0:a0 + 1, c0, :], pva[64:65, 0, :])
                            nc.vector.tensor_copy(nrm[a1:a1 + 1, c1, :], pva[64:65, 1, :])
                        else:
                            nc.vector.tensor_add(acc[0:64, p, :],
                                                 _f(acc[0:64, p, :]), pva[0:64, 0, :])
                            nc.vector.tensor_add(acc[64:128, p, :],
                                                 _f(acc[64:128, p, :]), pva[0:64, 1, :])
                            nc.vector.tensor_add(nrm[a0:a0 + 1, c0, :],
                                                 nrm[a0:a0 + 1, c0, :], pva[64:65, 0, :])
                            nc.vector.tensor_add(nrm[a1:a1 + 1, c1, :],
                                                 nrm[a1:a1 + 1, c1, :], pva[64:65, 1, :])


                # normalize: acc[:, p, :] *= 1/nrm via selector-matmul bcast
                aps.close()
                ppb = A.enter_context(tc.tile_pool(name="ppb", bufs=2, space="PSUM"))
                nc.vector.reciprocal(nrm[:], nrm[:])
                for p in range(8):
                    sel = sel_e if p % 2 == 0 else sel_o
                    ps_rb = ppb.tile([128, SQ], F32, name="ps_rb")
                    nc.tensor.matmul(ps_rb[:], sel, nrm[:, p // 2, :],
                                     start=True, stop=True)
                    nc.vector.tensor_mul(acc[:, p, :], _f(acc[:, p, :]), ps_rb[:])

            # ---- Stage 3: Wo + add1 + LN1 + residual (weights tag-reuse
            # ---- wk/wv/xsc slots so the DMAs run during attention) ----
            with ExitStack() as st:
                wo_sb = pwkv.tile([128, 8, D], DT, name="wo", tag="wk")
                for do in range(8):
                    nc.sync.dma_start(wo_sb[:, :, do * 128:(do + 1) * 128],
                                      r_wo[:, :, do * 128:(do + 1) * 128])
                aw1_sb = pwkv.tile([128, 8, D], DT, name="aw1", tag="wv")
                for do in range(8):
                    nc.sync.dma_start(aw1_sb[:, :, do * 128:(do + 1) * 128],
                                      r_aw1[:, :, do * 128:(do + 1) * 128])
                xq_sb = pxsc.tile([128, 8, SQ], DT, name="xq2", tag="xsc")
                for d in range(8):
                    nc.sync.dma_start(xq_sb[:, d, :], r_xq[:, d, :])
                pw = st.enter_context(tc.tile_pool(name="pw3", bufs=1))
                b1_sb = pw.tile([128, 8], F32, name="b1")
                nc.sync.dma_start(b1_sb[:], d_b1.ap())
                g1_sb = pw.tile([128, 8], F32, name="g1")
                nc.sync.dma_start(g1_sb[:], d_g1.ap())
                bb1_sb = pw.tile([128, 8], F32, name="bb1")
                nc.sync.dma_start(bb1_sb[:], d_bb1.ap())

                x1 = p_x1.tile([128, 8, SQ], DT, name="x1")
                pao = st.enter_context(tc.tile_pool(name="pao", bufs=1))
                ao = pao.tile([128, 8, SQ], DT, name="ao")
                pp = st.enter_context(tc.tile_pool(name="pp3", bufs=2, space="PSUM"))
                for do in range(8):
                    ps = pp.tile([128, SQ], F32, name="ps3a")
                    for d in range(8):
                        _mm(nc, ps[:], wo_sb[:, d, do * 128:(do + 1) * 128],
                            acc[:, d, :], start=(d == 0), stop=(d == 7))
                    nc.scalar.copy(ao[:, do, :], ps[:])
                l1 = pao.tile([128, 8, SQ], DT, name="l1")
                for do in range(8):
                    ps = pp.tile([128, SQ], F32, name="ps3b")
                    for d in range(8):
                        _mm(nc, ps[:], aw1_sb[:, d, do * 128:(do + 1) * 128],
                            ao[:, d, :], start=(d == 0), stop=(d == 7))
                    nc.vector.tensor_scalar(l1[:, do, :], ps[:],
                                            b1_sb[:, do:do + 1], None, OP.add)
                layernorm_block(st, l1, g1_sb, bb1_sb, xq_sb, x1, "a")

        # ================= Stage 4: FFN + add2 + LN2 + residual =================
        with ExitStack() as st:
            pff = st.enter_context(tc.tile_pool(name="pff", bufs=1))
            ff = pff.tile([128, 8, SQ], DT, name="ff")
            aw2_sb = pff.tile([128, 8, D], DT, name="aw2")
            for do in range(8):
                nc.sync.dma_start(aw2_sb[:, :, do * 128:(do + 1) * 128],
                                  r_aw2[:, :, do * 128:(do + 1) * 128])
            with ExitStack() as st4a:
                ph = st4a.enter_context(tc.tile_pool(name="ph", bufs=1))
                h_sb = ph.tile([128, 32, SQ], DT, name="h")
                pwc = st4a.enter_context(tc.tile_pool(name="pwc", bufs=6))
                pwc2 = st4a.enter_context(tc.tile_pool(name="pwc2", bufs=2))
                fb1_sb = ph.tile([128, 32], F32, name="fb1")
                nc.sync.dma_start(fb1_sb[:], d_fb1.ap())
                fb2_sb = ph.tile([128, 8], F32, name="fb2")
                nc.sync.dma_start(fb2_sb[:], d_fb2.ap())
                pp = st4a.enter_context(tc.tile_pool(name="pp4", bufs=2, space="PSUM"))

                for f in range(32):
                    w1t = pwc.tile([128, 8, 128], DT, name="w1c")
                    nc.sync.dma_start(w1t[:], r_fw1[:, :, f * 128:(f + 1) * 128])
                    ps = pp.tile([128, SQ], F32, name="ps4a")
                    for d in range(8):
                        _mm(nc, ps[:], w1t[:, d, :], x1[:, d, :],
                            start=(d == 0), stop=(d == 7))
                    nc.vector.tensor_scalar(h_sb[:, f, :], ps[:],
                                            fb1_sb[:, f:f + 1], 0.0,
                                            OP.add, OP.max)

                for do in range(8):
                    w2t = pwc2.tile([128, 32, 128], DT, name="w2c")
                    nc.sync.dma_start(w2t[:], r_fw2[:, :, do * 128:(do + 1) * 128])
                    ps = pp.tile([128, SQ], F32, name="ps4b")
                    for f in range(32):
                        _mm(nc, ps[:], w2t[:, f, :], h_sb[:, f, :],
                            start=(f == 0), stop=(f == 31))
                    nc.vector.tensor_scalar(ff[:, do, :], ps[:],
                                            fb2_sb[:, do:do + 1], None, OP.add)

            with ExitStack() as st4b:
                pw = st4b.enter_context(tc.tile_pool(name="pw4", bufs=1))
                b2_sb = pw.tile([128, 8], F32, name="b2")
                nc.sync.dma_start(b2_sb[:], d_b2.ap())
                g2_sb = pw.tile([128, 8], F32, name="g2")
                nc.sync.dma_start(g2_sb[:], d_g2.ap())
                bb2_sb = pw.tile([128, 8], F32, name="bb2")
                nc.sync.dma_start(bb2_sb[:], d_bb2.ap())
                pp = st4b.enter_context(tc.tile_pool(name="pp4b", bufs=2, space="PSUM"))

                l2 = pw.tile([128, 8, SQ], DT, name="l2")
                for do in range(8):
                    ps = pp.tile([128, SQ], F32, name="ps4c")
                    for d in range(8):
                        _mm(nc, ps[:], aw2_sb[:, d, do * 128:(do + 1) * 128],
                            ff[:, d, :], start=(d == 0), stop=(d == 7))
                    nc.vector.tensor_scalar(l2[:, do, :], ps[:],
                                            b2_sb[:, do:do + 1], None, OP.add)

                outp = pw.tile([128, 8, SQ], F32, name="outp")
                layernorm_block(st4b, l2, g2_sb, bb2_sb, x1, outp, "b")
                nc.sync.dma_start(r_out, outp[:])

    nc.compile()
    return nc


_NC = None


def _get_nc():
    global _NC
    if _NC is None:
        _NC = build_nc()
    return _NC


def _prep_inputs(inputs):
    """Host-side shard prep: per-core input dicts."""
    x = np.asarray(inputs["batch_x"], np.float32)       # [B, S, D]
    lens = np.asarray(inputs["len_chair"], np.int64)
    wq = np.ascontiguousarray(
        np.asarray(inputs["Wq"], np.float32).transpose(1, 0, 2).reshape(D, D))
    wk = np.ascontiguousarray(
        np.asarray(inputs["Wk"], np.float32).transpose(1, 0, 2).reshape(D, D))
    wv = np.ascontiguousarray(
        np.asarray(inputs["Wv"], np.float32).transpose(1, 0, 2).reshape(D, D))
    com = {
        "wq": wq, "wk": wk, "wv": wv,
        "wo": np.ascontiguousarray(np.asarray(inputs["Wo"], np.float32)),
        "aw1": np.ascontiguousarray(np.asarray(inputs["add1_w"], np.float32)),
        "aw2": np.ascontiguousarray(np.asarray(inputs["add2_w"], np.float32)),
        "fw1": np.ascontiguousarray(np.asarray(inputs["ff_w1"], np.float32)),
        "fw2": np.ascontiguousarray(np.asarray(inputs["ff_w2"], np.float32)),
        "b1c": _chunk(inputs["add1_b"]), "g1c": _chunk(inputs["ln1_g"]),
        "bb1c": _chunk(inputs["ln1_b"]), "fb1c": _chunk(inputs["ff_b1"]),
        "fb2c": _chunk(inputs["ff_b2"]), "b2c": _chunk(inputs["add2_b"]),
        "g2c": _chunk(inputs["ln2_g"]), "bb2c": _chunk(inputs["ln2_b"]),
    }
    xT = [np.ascontiguousarray(x[b].T) for b in range(B)]   # [D, S]
    masks = []
    for b in range(B):
        m = np.where(np.arange(S) >= lens[b], np.float32(MASK_NEG),
                     np.float32(0.0)).astype(np.float32)
        masks.append(np.ascontiguousarray(m.reshape(S // 128, 128).T))
    in_maps = []
    for c in range(NCORES):
        b, q = c // 4, c % 4
        m = dict(com)
        m["xT"] = xT[b]
        m["xq"] = np.ascontiguousarray(xT[b][:, q * SQ:(q + 1) * SQ])
        m["maskb"] = masks[b]
        in_maps.append(m)
    return in_maps


def _chunk(v):
    v = np.asarray(v, np.float32)
    return np.ascontiguousarray(v.reshape(-1, 128).T)


DT_KEYS = ("xT", "xq", "wq", "wk", "wv", "wo", "aw1", "aw2", "fw1", "fw2")
BF16_KEYS = ()


def kernel(trace=False, **inputs):
    _install_ntff_hook()
    nc = _get_nc()
    in_maps = _prep_inputs(inputs)
    import ml_dtypes
    np_dt = mybir.dt.np(DT)
    cache = {}

    def _cast(a, dtype):
        key = (id(a), np.dtype(dtype).str)
        if key not in cache:
            cache[key] = np.ascontiguousarray(a.astype(dtype))
        return cache[key]

    for m in in_maps:
        for k in BF16_KEYS:
            m[k] = _cast(m[k], ml_dtypes.bfloat16)
        if np_dt != np.float32:
            for k in DT_KEYS:
                m[k] = _cast(m[k], np_dt)
    res = run_bass_kernel_spmd(nc, in_maps, core_ids=list(range(NCORES)),
                               trace=trace)
    out = np.empty((B, S, D), np.float32)
    for c in range(NCORES):
        b, q = c // 4, c % 4
        out[b, q * SQ:(q + 1) * SQ, :] = res.results[c]["out"].T
    kernel.last_exec_time_ns = res.exec_time_ns
    return out



# revision 7
# speedup vs baseline: 1.0593x; 1.0593x over previous
"""Trainium2 Bass kernel for nn_Encoder_Block (B=2,S=2048,D=1024,H=16,FF=4096).

Sharding: 8 cores, core c -> (batch b=c//4, query block q=c%4 of 512 tokens).
Each core recomputes K/V for its whole batch (no cross-core collectives),
everything else is perfectly sharded. Host does transposes and gather.

Device layout: activations kept transposed [feature, token] throughout, so
every matmul in the chain is a natural lhsT/rhs pair with K=128 contraction
chunks and N=512 moving dim. Attention computes transposed scores [t, sq];
softmax normalizer rides along the PV matmul as a ones-column in V (M=65).
Masking + 1/sqrt(dh) scaling are folded into the Exp activation (bias/scale).
No max-subtraction: scores are O(1) by construction, exp is safe.

Perf notes (round 1):
- Fully-masked key chunks (j*128 >= len) contribute exactly 0 to both the
  softmax numerator and denominator, so the program only processes
  NCH = max_b ceil(len_b/128) chunks (compiled per NCH, cached).
- bf16 matmul dtype: same PE rate as f32r but half DMA bytes, FWL weight
  loads, 2x DVE; rel err ~5e-3 vs the 2e-2 gate.
- DMA issue spread across the three queues (sync/SP + scalar/ACT HWDGE +
  gpsimd SWDGE) so weight prefetch never blocks the critical queue.
- psum->SBUF copies moved off ACT (exp no longer stalls PV).
- LayerNorm: DVE bf16 sub/mul, ACT does *g+b, gpsimd adds the residual.
"""
import sys, types, os
sys.path.insert(0, "/opt/trn_rl_repo")
import numpy as np
from contextlib import ExitStack

import concourse.bass as bass
import concourse.tile as tile
from concourse import bacc, mybir
from concourse.bass_utils import run_bass_kernel_spmd

B, S, D, H, FF = 2, 2048, 1024, 16, 4096
DH = D // H            # 64
SQ = 512               # query tokens per core
NCORES = 8
EPS = 1e-5
MASK_NEG = -60.0       # exp(-60) ~ 8.8e-27 => masked keys contribute ~0

F32 = mybir.dt.float32
# PE compute dtype for matmul-fed tensors:
#   bf16 : 1 cyc/row, half DMA/LDWEIGHTS cost, ~5e-3 output error
#   f32r : 1 cyc/row TF32-like, ~3.5e-4 output error
_MODE = os.environ.get("KERNEL_MM_DT", "bf16")
DT = {"f32r": mybir.dt.float32r, "f32": mybir.dt.float32,
      "bf16": mybir.dt.bfloat16}[_MODE]


def _f(ap):
    # f32r tiles must be bitcast to f32 for DVE/ACT reads; bf16 is native
    if DT == mybir.dt.float32r:
        return ap.bitcast(F32)
    return ap


def _install_ntff_hook():
    """The image's antenv lacks axon_hooks; shim it so trace=True works."""
    try:
        import antenv.axon_hooks  # noqa
        return
    except ImportError:
        pass
    try:
        from trn_agent_boot.trn_boot import _ntff_profile_via_ctypes
        import antenv
        mod = types.ModuleType("antenv.axon_hooks")
        hook = _ntff_profile_via_ctypes("/opt/axon/libaxon_pjrt.so")
        mod.get_axon_ntff_profile_hook = lambda: hook
        mod.set_axon_ntff_profile_hook = lambda h: None
        sys.modules["antenv.axon_hooks"] = mod
        antenv.axon_hooks = mod
    except Exception:
        pass


def _mm(nc, out, lhsT, rhs, start, stop, tile_position=None):
    nc.tensor.matmul(out, lhsT, rhs,
                     start=start, stop=stop, tile_position=tile_position)


def build_nc(nch):
    """nch = number of active 128-key chunks (<= 16)."""
    nc = bacc.Bacc(trn_type="TRN2", target_bir_lowering=False, debug=False,
                   num_devices=NCORES, dynamic_dma_scratch_size=512)
    AF = mybir.ActivationFunctionType
    OP = mybir.AluOpType

    # super-chunks of up to 4 chunks (512 keys)
    scs = []
    c0 = 0
    while c0 < nch:
        scs.append((c0, min(4, nch - c0)))
        c0 += min(4, nch - c0)

    # ---- DRAM I/O (per-core; program identical across cores) ----
    d_xT = nc.dram_tensor("xT", [D, S], DT, kind="ExternalInput")
    d_xq = nc.dram_tensor("xq", [D, SQ], DT, kind="ExternalInput")
    d_mask = nc.dram_tensor("maskb", [128, S // 128], F32, kind="ExternalInput")
    d_wq = nc.dram_tensor("wq", [D, D], DT, kind="ExternalInput")
    d_wk = nc.dram_tensor("wk", [D, D], DT, kind="ExternalInput")
    d_wv = nc.dram_tensor("wv", [D, D], DT, kind="ExternalInput")
    d_wo = nc.dram_tensor("wo", [D, D], DT, kind="ExternalInput")
    d_aw1 = nc.dram_tensor("aw1", [D, D], DT, kind="ExternalInput")
    d_aw2 = nc.dram_tensor("aw2", [D, D], DT, kind="ExternalInput")
    d_fw1 = nc.dram_tensor("fw1", [D, FF], DT, kind="ExternalInput")
    d_fw2 = nc.dram_tensor("fw2", [FF, D], DT, kind="ExternalInput")
    d_b1 = nc.dram_tensor("b1c", [128, 8], F32, kind="ExternalInput")
    d_g1 = nc.dram_tensor("g1c", [128, 8], F32, kind="ExternalInput")
    d_bb1 = nc.dram_tensor("bb1c", [128, 8], F32, kind="ExternalInput")
    d_fb1 = nc.dram_tensor("fb1c", [128, 32], F32, kind="ExternalInput")
    d_fb2 = nc.dram_tensor("fb2c", [128, 8], F32, kind="ExternalInput")
    d_b2 = nc.dram_tensor("b2c", [128, 8], F32, kind="ExternalInput")
    d_g2 = nc.dram_tensor("g2c", [128, 8], F32, kind="ExternalInput")
    d_bb2 = nc.dram_tensor("bb2c", [128, 8], F32, kind="ExternalInput")
    d_out = nc.dram_tensor("out", [D, SQ], F32, kind="ExternalOutput")

    r_xT = d_xT.ap().rearrange("(c p) s -> p c s", p=128)     # [128, 8, S]
    r_xq = d_xq.ap().rearrange("(c p) s -> p c s", p=128)     # [128, 8, SQ]
    r_wq = d_wq.ap().rearrange("(c p) n -> p c n", p=128)
    r_wk = d_wk.ap().rearrange("(c p) n -> p c n", p=128)
    r_wv = d_wv.ap().rearrange("(c p) n -> p c n", p=128)
    r_wo = d_wo.ap().rearrange("(c p) n -> p c n", p=128)
    r_aw1 = d_aw1.ap().rearrange("(c p) n -> p c n", p=128)
    r_aw2 = d_aw2.ap().rearrange("(c p) n -> p c n", p=128)
    r_fw1 = d_fw1.ap().rearrange("(c p) n -> p c n", p=128)   # [128, 8, FF]
    r_fw2 = d_fw2.ap().rearrange("(c p) n -> p c n", p=128)   # [128, 32, D]
    r_out = d_out.ap().rearrange("(c p) s -> p c s", p=128)

    with tile.TileContext(nc) as tc:
      with ExitStack() as top:
        # ---- persistent pools & early DMA issue --------------------------
        # stage-1a Q-proj inputs go FIRST on the sync queue so the PE can
        # start within a few us of kernel entry.
        p_x1 = top.enter_context(tc.tile_pool(name="px1", bufs=1))
        x1 = p_x1.tile([128, 8, SQ], DT, name="x1")
        xq_sb = p_x1.tile([128, 8, SQ], DT, name="xqp")
        for d in range(8):
            nc.sync.dma_start(xq_sb[:, d, :], r_xq[:, d, :])

        # one packed const tile (tiles pad to 4KB/partition each otherwise):
        # cols 0:16 maskbias, 16:80 ones, 80:208 sel_e, 208:336 sel_o
        const = top.enter_context(tc.tile_pool(name="const", bufs=1))
        cst = const.tile([128, 336], F32, name="cst")
        mask_sb = cst[:, 0:16]
        ones_f = cst[:, 16:80]
        sel_e = cst[:, 80:208]
        sel_o = cst[:, 208:336]
        nc.scalar.dma_start(mask_sb, d_mask.ap())
        nc.vector.memset(cst[:, 16:336], 0.0)
        nc.vector.memset(ones_f, 1.0)
        nc.vector.memset(sel_e[0:1, 0:64], 1.0)
        nc.vector.memset(sel_e[32:33, 64:128], 1.0)
        nc.vector.memset(sel_o[64:65, 0:64], 1.0)
        nc.vector.memset(sel_o[96:97, 64:128], 1.0)
        ones_sb = const.tile([128, 1], DT, name="ones")
        nc.vector.tensor_copy(ones_sb[:], ones_f[:, 0:1])

        # LN/bias coefficient tiles + first fw1 chunks on the gpsimd SWDGE
        # queue (gpsimd has no other work until LN1 ~350us in).
        pcoef = top.enter_context(tc.tile_pool(name="pcoef", bufs=1))
        b1_sb = pcoef.tile([128, 8], F32, name="b1")
        g1_sb = pcoef.tile([128, 8], F32, name="g1")
        bb1_sb = pcoef.tile([128, 8], F32, name="bb1")
        fb1_sb = pcoef.tile([128, 32], F32, name="fb1")
        fb2_sb = pcoef.tile([128, 8], F32, name="fb2")
        b2_sb = pcoef.tile([128, 8], F32, name="b2")
        g2_sb = pcoef.tile([128, 8], F32, name="g2")
        bb2_sb = pcoef.tile([128, 8], F32, name="bb2")
        for t_sb, t_d in ((b1_sb, d_b1), (g1_sb, d_g1), (bb1_sb, d_bb1),
                          (fb1_sb, d_fb1), (fb2_sb, d_fb2), (b2_sb, d_b2),
                          (g2_sb, d_g2), (bb2_sb, d_bb2)):
            nc.gpsimd.dma_start(t_sb[:], t_d.ap())

        # fw1 prefetch slots; DMAs are issued on sync once the startup
        # loads are past (at super-chunk 1 in the attention loop below).
        NPRE = 6
        pwc = top.enter_context(tc.tile_pool(name="pwc", bufs=NPRE))
        w1pre = []
        for f in range(NPRE):
            w1t = pwc.tile([128, 8, 128], DT, name="w1c", tag="w1c")
            w1pre.append(w1t)

        def emit_w1pre():
            for f, w1t in enumerate(w1pre):
                nc.sync.dma_start(w1t[:], r_fw1[:, :, f * 128:(f + 1) * 128])

        def layernorm_block(st, src_sb, gc, bc, res_sb, dst_sb, pref,
                            out_dram=None):
            """dst = LN(src) * g + b + res, all [128, 8, SQ] chunked over D.

            DVE does (x-mu)*rstd in bf16, ACT applies the per-partition
            affine (*g + b), gpsimd adds the residual; the four engines
            pipeline across d-chunks.  If out_dram is given, each finished
            d-chunk is DMA'd out immediately (sync queue).
            """
            pln = st.enter_context(tc.tile_pool(name=pref + "ln", bufs=1))
            pps = st.enter_context(tc.tile_pool(name=pref + "lps", bufs=1, space="PSUM"))
            sq_sb = pln.tile([128, 8, SQ], DT, name=pref + "sq")
            for d in range(8):
                nc.vector.tensor_mul(sq_sb[:, d, :], _f(src_sb[:, d, :]),
                                     _f(src_sb[:, d, :]))
            ps_s = pps.tile([1, SQ], F32, name=pref + "ps_s")
            ps_q = pps.tile([1, SQ], F32, name=pref + "ps_q")
            for d in range(8):
                _mm(nc, ps_s[:], ones_sb[:], src_sb[:, d, :],
                    start=(d == 0), stop=(d == 7))
            for d in range(8):
                _mm(nc, ps_q[:], ones_sb[:], sq_sb[:, d, :],
                    start=(d == 0), stop=(d == 7))
            mu = pln.tile([1, SQ], F32, name=pref + "mu")
            nc.scalar.mul(mu[:], ps_s[:], 1.0 / D)
            msq = pln.tile([1, SQ], F32, name=pref + "msq")
            nc.scalar.mul(msq[:], ps_q[:], 1.0 / D)
            var = pln.tile([1, SQ], F32, name=pref + "var")
            nc.vector.tensor_mul(var[:], mu[:], mu[:])
            nc.vector.tensor_sub(var[:], msq[:], var[:])
            nc.vector.tensor_scalar_add(var[:], var[:], EPS)
            # rstd = exp(-0.5*ln(var)): Ln/Exp share the ACT table set with
            # the attention exps (no sqrt-table load).
            lnv = pln.tile([1, SQ], F32, name=pref + "lnv")
            nc.scalar.activation(lnv[:], var[:], AF.Ln)
            rstd = pln.tile([1, SQ], DT, name=pref + "rstd")
            nc.scalar.activation(rstd[:], lnv[:], AF.Exp, scale=-0.5)
            mu_h = pln.tile([1, SQ], DT, name=pref + "muh")
            nc.vector.tensor_copy(mu_h[:], mu[:])
            mub = pln.tile([128, SQ], DT, name=pref + "mub")
            rsb = pln.tile([128, SQ], DT, name=pref + "rsb")
            nc.gpsimd.partition_broadcast(mub[:], mu_h[:])
            nc.gpsimd.partition_broadcast(rsb[:], rstd[:])
            tmp = pln.tile([128, 8, SQ], DT, name=pref + "tmp")
            tmp2 = pln.tile([128, 8, SQ], DT, name=pref + "tmp2")
            for d in range(8):
                nc.vector.tensor_sub(tmp[:, d, :], _f(src_sb[:, d, :]), mub[:])
                nc.vector.tensor_mul(tmp[:, d, :], tmp[:, d, :], rsb[:])
                nc.scalar.activation(tmp2[:, d, :], tmp[:, d, :], AF.Identity,
                                     bias=bc[:, d:d + 1], scale=gc[:, d:d + 1])
                nc.gpsimd.tensor_add(dst_sb[:, d, :], tmp2[:, d, :],
                                     _f(res_sb[:, d, :]))
                if out_dram is not None:
                    nc.sync.dma_start(out_dram[:, d, :], dst_sb[:, d, :])

        # ============ Stages 1-3 ============
        with ExitStack() as s13:
            # weights: wq first on sync (Q proj is the kernel's first PE
            # work); wk/wv then wo/aw1 on the scalar HWDGE queue, which is
            # idle until the first exp ~18us in.
            pwkv = s13.enter_context(tc.tile_pool(name="pwkv", bufs=1))
            wq_sb = pwkv.tile([128, 8, D], DT, name="wq")
            for d in range(8):
                nc.sync.dma_start(wq_sb[:, d, :], r_wq[:, d, :])
            wk_sb = pwkv.tile([128, 8, D], DT, name="wk")
            wv_sb = pwkv.tile([128, 8, D], DT, name="wv")
            wo_sb = pwkv.tile([128, 8, D], DT, name="wo")
            aw1_sb = pwkv.tile([128, 8, D], DT, name="aw1")
            for h in range(2):
                nc.scalar.dma_start(wk_sb[:, 4 * h:4 * h + 4, :],
                                    r_wk[:, 4 * h:4 * h + 4, :])
                nc.scalar.dma_start(wv_sb[:, 4 * h:4 * h + 4, :],
                                    r_wv[:, 4 * h:4 * h + 4, :])
            for h in range(2):
                nc.scalar.dma_start(wo_sb[:, 4 * h:4 * h + 4, :],
                                    r_wo[:, 4 * h:4 * h + 4, :])
                nc.scalar.dma_start(aw1_sb[:, 4 * h:4 * h + 4, :],
                                    r_aw1[:, 4 * h:4 * h + 4, :])

            p_acc = s13.enter_context(tc.tile_pool(name="acc", bufs=1))
            acc = p_acc.tile([128, 8, SQ], DT, name="acc")
            # softmax denominators at partition 32*(h%4), free idx h//4;
            # init 1.0 so unused rows stay finite through reciprocal+selector
            nrm = p_acc.tile([128, 4, SQ], F32, name="nrm")
            nc.vector.memset(nrm[:], 1.0)

            pxsc = s13.enter_context(tc.tile_pool(name="pxsc", bufs=1))

            with ExitStack() as A:
                p_qT = A.enter_context(tc.tile_pool(name="qT", bufs=1))
                qT = p_qT.tile([128, 8, SQ], DT, name="qT")

                # ---- Stage 1a: Q^T projection ----
                with ExitStack() as st:
                    pp = st.enter_context(tc.tile_pool(name="ppq", bufs=2, space="PSUM"))
                    for p in range(8):
                        ps = pp.tile([128, SQ], F32, name="psq")
                        for d in range(8):
                            _mm(nc, ps[:], wq_sb[:, d, p * 128:(p + 1) * 128],
                                xq_sb[:, d, :], start=(d == 0), stop=(d == 7))
                        nc.vector.tensor_copy(qT[:, p, :], ps[:])

                # ---- Stage 1b+2: K/V proj + attention, flash over chunks ----
                pkv = A.enter_context(tc.tile_pool(name="pkv", bufs=2))
                pexp = A.enter_context(tc.tile_pool(name="pexp", bufs=3))
                aps = A.enter_context(ExitStack())
                psc = aps.enter_context(tc.tile_pool(name="psc", bufs=2, space="PSUM"))
                pp = psc
                ppv = aps.enter_context(tc.tile_pool(name="ppv", bufs=2, space="PSUM"))

                for isc, (t0c, ntc) in enumerate(scs):
                    if isc == 1:
                        emit_w1pre()
                    t0 = t0c * 128
                    nk = ntc * 128
                    xs = pxsc.tile([128, 8, 512], DT, name="xsc", tag="xsc")
                    for d in range(8):
                        nc.sync.dma_start(xs[:, d, 0:nk], r_xT[:, d, t0:t0 + nk])

                    kT = pkv.tile([128, 8, 512], DT, name="kT")
                    for p in range(8):
                        ps = pp.tile([128, 2, SQ], F32, name="s01")[:, 0, 0:nk]
                        for d in range(8):
                            _mm(nc, ps, wk_sb[:, d, p * 128:(p + 1) * 128],
                                xs[:, d, 0:nk], start=(d == 0), stop=(d == 7))
                        nc.vector.tensor_copy(kT[:, p, 0:nk], ps)

                    vt = pkv.tile([128, 4, 16, 65], DT, name="vt")
                    nc.vector.tensor_copy(
                        vt[:, 0:ntc, :, 64:65],
                        ones_f.rearrange("p (a b c) -> p a b c",
                                         a=4, b=16)[:, 0:ntc])
                    for i in range(ntc):
                        for nb in range(2):
                            ps = pp.tile([128, 2, SQ], F32, name="s01")[:, 0, :]
                            for d in range(8):
                                _mm(nc, ps, xs[:, d, i * 128:(i + 1) * 128],
                                    wv_sb[:, d, nb * 512:(nb + 1) * 512],
                                    start=(d == 0), stop=(d == 7))
                            nc.vector.tensor_copy(
                                vt[:, i, nb * 8:(nb + 1) * 8, 0:64],
                                ps.rearrange("p (h e) -> p h e", e=64))

                    for p in range(8):
                        h0, h1 = 2 * p, 2 * p + 1
                        pva = ppv.tile([128, 2, SQ], F32, name="pva")
                        for i in range(ntc):
                            tci = t0c + i
                            s01 = psc.tile([128, 2, SQ], F32, name="s01")
                            _mm(nc, s01[:, 0, :],
                                kT[0:64, p, i * 128:(i + 1) * 128],
                                qT[0:64, p, :], start=True, stop=True,
                                tile_position=(0, 0))
                            _mm(nc, s01[:, 1, :],
                                kT[64:128, p, i * 128:(i + 1) * 128],
                                qT[64:128, p, :], start=True, stop=True,
                                tile_position=(64, 0))
                            e01 = pexp.tile([128, 2, SQ], DT, name="e01")
                            nc.scalar.activation(e01[:], s01[:], AF.Exp,
                                                 bias=mask_sb[:, tci:tci + 1],
                                                 scale=0.125)
                            _mm(nc, pva[0:65, 0, :], vt[:, i, h0, :], e01[:, 0, :],
                                start=(i == 0), stop=(i == ntc - 1))
                            _mm(nc, pva[0:65, 1, :], vt[:, i, h1, :], e01[:, 1, :],
                                start=(i == 0), stop=(i == ntc - 1))
                        a0, c0 = 32 * (h0 % 4), h0 // 4
                        a1, c1 = 32 * (h1 % 4), h1 // 4
                        if isc == 0:
                            nc.vector.tensor_copy(acc[0:64, p, :], pva[0:64, 0, :])
                            nc.vector.tensor_copy(acc[64:128, p, :], pva[0:64, 1, :])
                            nc.vector.tensor_copy(nrm[a0:a0 + 1, c0, :], pva[64:65, 0, :])
                            nc.vector.tensor_copy(nrm[a1:a1 + 1, c1, :], pva[64:65, 1, :])
                        else:
                            nc.vector.tensor_add(acc[0:64, p, :],
                                                 _f(acc[0:64, p, :]), pva[0:64, 0, :])
                            nc.vector.tensor_add(acc[64:128, p, :],
                                                 _f(acc[64:128, p, :]), pva[0:64, 1, :])
                            nc.vector.tensor_add(nrm[a0:a0 + 1, c0, :],
                                                 nrm[a0:a0 + 1, c0, :], pva[64:65, 0, :])
                            nc.vector.tensor_add(nrm[a1:a1 + 1, c1, :],
                                                 nrm[a1:a1 + 1, c1, :], pva[64:65, 1, :])

                if len(scs) == 1:
                    emit_w1pre()
                # normalize: acc[:, p, :] *= 1/nrm via selector-matmul bcast
                aps.close()
                ppb = A.enter_context(tc.tile_pool(name="ppb", bufs=2, space="PSUM"))
                nc.vector.reciprocal_approx_fast(nrm[:], nrm[:])
                for p in range(8):
                    sel = sel_e if p % 2 == 0 else sel_o
                    ps_rb = ppb.tile([128, SQ], F32, name="ps_rb")
                    nc.tensor.matmul(ps_rb[:], sel, nrm[:, p // 2, :],
                                     start=True, stop=True)
                    nc.vector.tensor_mul(acc[:, p, :], _f(acc[:, p, :]), ps_rb[:])

            # ---- Stage 3: Wo + add1 + LN1 + residual ----
            with ExitStack() as st:
                pao = st.enter_context(tc.tile_pool(name="pao", bufs=1))
                ao = pao.tile([128, 8, SQ], DT, name="ao")
                pp = st.enter_context(tc.tile_pool(name="pp3", bufs=2, space="PSUM"))
                for do in range(8):
                    ps = pp.tile([128, SQ], F32, name="ps3a")
                    for d in range(8):
                        _mm(nc, ps[:], wo_sb[:, d, do * 128:(do + 1) * 128],
                            acc[:, d, :], start=(d == 0), stop=(d == 7))
                    nc.vector.tensor_copy(ao[:, do, :], ps[:])
                l1 = pao.tile([128, 8, SQ], DT, name="l1")
                for do in range(8):
                    ps = pp.tile([128, SQ], F32, name="ps3b")
                    for d in range(8):
                        _mm(nc, ps[:], aw1_sb[:, d, do * 128:(do + 1) * 128],
                            ao[:, d, :], start=(d == 0), stop=(d == 7))
                    nc.vector.tensor_scalar(l1[:, do, :], ps[:],
                                            b1_sb[:, do:do + 1], None, OP.add)
                layernorm_block(st, l1, g1_sb, bb1_sb, xq_sb, x1, "a")

        # ================= Stage 4: FFN + add2 + LN2 + residual =================
        with ExitStack() as st:
            pff = st.enter_context(tc.tile_pool(name="pff", bufs=1))
            ff = pff.tile([128, 8, SQ], DT, name="ff")
            aw2_sb = pff.tile([128, 8, D], DT, name="aw2")
            for h in range(2):
                nc.scalar.dma_start(aw2_sb[:, 4 * h:4 * h + 4, :],
                                    r_aw2[:, 4 * h:4 * h + 4, :])
            with ExitStack() as st4a:
                ph = st4a.enter_context(tc.tile_pool(name="ph", bufs=1))
                h_sb = ph.tile([128, 32, SQ], DT, name="h")
                pwc2 = st4a.enter_context(tc.tile_pool(name="pwc2", bufs=2))
                pp = st4a.enter_context(tc.tile_pool(name="pp4", bufs=2, space="PSUM"))

                for f in range(32):
                    if f < NPRE:
                        w1t = w1pre[f]
                    else:
                        w1t = pwc.tile([128, 8, 128], DT, name="w1c", tag="w1c")
                        nc.sync.dma_start(w1t[:], r_fw1[:, :, f * 128:(f + 1) * 128])
                    ps = pp.tile([128, SQ], F32, name="ps4a")
                    for d in range(8):
                        _mm(nc, ps[:], w1t[:, d, :], x1[:, d, :],
                            start=(d == 0), stop=(d == 7))
                    nc.vector.tensor_scalar(h_sb[:, f, :], ps[:],
                                            fb1_sb[:, f:f + 1], 0.0,
                                            OP.add, OP.max)

                for do in range(8):
                    w2t = pwc2.tile([128, 32, 128], DT, name="w2c")
                    nc.sync.dma_start(w2t[:], r_fw2[:, :, do * 128:(do + 1) * 128])
                    ps = pp.tile([128, SQ], F32, name="ps4b")
                    for f in range(32):
                        _mm(nc, ps[:], w2t[:, f, :], h_sb[:, f, :],
                            start=(f == 0), stop=(f == 31))
                    nc.vector.tensor_scalar(ff[:, do, :], ps[:],
                                            fb2_sb[:, do:do + 1], None, OP.add)

            with ExitStack() as st4b:
                pw = st4b.enter_context(tc.tile_pool(name="pw4", bufs=1))
                pp = st4b.enter_context(tc.tile_pool(name="pp4b", bufs=2, space="PSUM"))

                l2 = pw.tile([128, 8, SQ], DT, name="l2")
                for do in range(8):
                    ps = pp.tile([128, SQ], F32, name="ps4c")
                    for d in range(8):
                        _mm(nc, ps[:], aw2_sb[:, d, do * 128:(do + 1) * 128],
                            ff[:, d, :], start=(d == 0), stop=(d == 7))
                    nc.vector.tensor_scalar(l2[:, do, :], ps[:],
                                            b2_sb[:, do:do + 1], None, OP.add)

                outp = pw.tile([128, 8, SQ], F32, name="outp")
                layernorm_block(st4b, l2, g2_sb, bb2_sb, x1, outp, "b",
                                out_dram=r_out)

    nc.compile()
    return nc


_NC_CACHE = {}


def _get_nc(nch):
    if nch not in _NC_CACHE:
        _NC_CACHE[nch] = build_nc(nch)
    return _NC_CACHE[nch]


def _prep_inputs(inputs):
    """Host-side shard prep: per-core input dicts."""
    x = np.asarray(inputs["batch_x"], np.float32)       # [B, S, D]
    lens = np.asarray(inputs["len_chair"], np.int64)
    wq = np.ascontiguousarray(
        np.asarray(inputs["Wq"], np.float32).transpose(1, 0, 2).reshape(D, D))
    wk = np.ascontiguousarray(
        np.asarray(inputs["Wk"], np.float32).transpose(1, 0, 2).reshape(D, D))
    wv = np.ascontiguousarray(
        np.asarray(inputs["Wv"], np.float32).transpose(1, 0, 2).reshape(D, D))
    com = {
        "wq": wq, "wk": wk, "wv": wv,
        "wo": np.ascontiguousarray(np.asarray(inputs["Wo"], np.float32)),
        "aw1": np.ascontiguousarray(np.asarray(inputs["add1_w"], np.float32)),
        "aw2": np.ascontiguousarray(np.asarray(inputs["add2_w"], np.float32)),
        "fw1": np.ascontiguousarray(np.asarray(inputs["ff_w1"], np.float32)),
        "fw2": np.ascontiguousarray(np.asarray(inputs["ff_w2"], np.float32)),
        "b1c": _chunk(inputs["add1_b"]), "g1c": _chunk(inputs["ln1_g"]),
        "bb1c": _chunk(inputs["ln1_b"]), "fb1c": _chunk(inputs["ff_b1"]),
        "fb2c": _chunk(inputs["ff_b2"]), "b2c": _chunk(inputs["add2_b"]),
        "g2c": _chunk(inputs["ln2_g"]), "bb2c": _chunk(inputs["ln2_b"]),
    }
    xT = [np.ascontiguousarray(x[b].T) for b in range(B)]   # [D, S]
    masks = []
    for b in range(B):
        m = np.where(np.arange(S) >= lens[b], np.float32(MASK_NEG),
                     np.float32(0.0)).astype(np.float32)
        masks.append(np.ascontiguousarray(m.reshape(S // 128, 128).T))
    in_maps = []
    for c in range(NCORES):
        b, q = c // 4, c % 4
        m = dict(com)
        m["xT"] = xT[b]
        m["xq"] = np.ascontiguousarray(xT[b][:, q * SQ:(q + 1) * SQ])
        m["maskb"] = masks[b]
        in_maps.append(m)
    return in_maps


def _chunk(v):
    v = np.asarray(v, np.float32)
    return np.ascontiguousarray(v.reshape(-1, 128).T)


DT_KEYS = ("xT", "xq", "wq", "wk", "wv", "wo", "aw1", "aw2", "fw1", "fw2")


def kernel(trace=False, **inputs):
    _install_ntff_hook()
    lens = np.asarray(inputs["len_chair"], np.int64)
    nch = int(max(1, -(-int(lens.max()) // 128)))  # ceil(max_len/128)
    nch = min(nch, S // 128)
    nc = _get_nc(nch)
    in_maps = _prep_inputs(inputs)
    import ml_dtypes
    np_dt = mybir.dt.np(DT)
    cache = {}

    def _cast(a, dtype):
        key = (id(a), np.dtype(dtype).str)
        if key not in cache:
            cache[key] = np.ascontiguousarray(a.astype(dtype))
        return cache[key]

    for m in in_maps:
        if np_dt != np.float32:
            for k in DT_KEYS:
                m[k] = _cast(m[k], np_dt)
    res = run_bass_kernel_spmd(nc, in_maps, core_ids=list(range(NCORES)),
                               trace=trace)
    out = np.empty((B, S, D), np.float32)
    for c in range(NCORES):
        b, q = c // 4, c % 4
        out[b, q * SQ:(q + 1) * SQ, :] = res.results[c]["out"].T
    kernel.last_exec_time_ns = res.exec_time_ns
    return out
